# revision 43
# baseline (speedup 1.0000x reference)
"""Scatter-add (col2im at random query corners) on 8 Trainium2 NeuronCores.

Problem: out[t,c,h+dh,w+dw] += patches[n,0,c,dh,dw] for each query n at
corner (t,h,w), on top of the vid2fill base. PT=1, so every patch touches
exactly one frame: shard by frame pairs (core k owns frames 2k, 2k+1); the
cores are fully independent, no collective needed.

Strategy ("depth compaction", column-major): the host computes each output
element's contributor count (its depth d) and sorts device-handled
elements (d >= 2) by depth descending; element i lands at SBUF slot
(partition i%128, column i//128). Layer r of the accumulation then only
concerns the first W[r+1] columns (a prefix), so the device streams one
contiguous fp16 DMA load per layer block and performs one in-place
full-partition vector add per layer; column regions are stored as soon as
the layers covering them have folded. Every addition of the scatter-add
happens on-device as a dense, full-bandwidth op — the memory-regime
optimum (device traffic ~= fp16 patch bytes + output bytes; padding is
under one column per layer plus cross-core size spread).

The deepest columns [0, X) are accumulated by a parallel gpsimd (Pool)
chain over all layers while the DVE chain handles [X, W[l]); the ranges
are column-disjoint so the engines never race, and the slow serial tail
of tiny deep layers runs concurrently with the wide DVE adds instead of
gating the final store. fp16 halves DMA traffic vs f32 (the 2e-2
tolerance leaves ~16x margin over fp16's ~1.2e-3 worst case); trailing
small layer loads are merged into one block and the deepest store spans
>= 256 columns so every DMA keeps >= 512 contiguous bytes per descriptor
(below that the DMA bus pays a 2x latency multiplier).

Elements with depth 0 (base only) and depth 1 (a single contribution, no
addition required anywhere) are routed by the host during unpermutation at
full f32 precision.
"""

import sys
from contextlib import ExitStack

for _p in ("/opt/trn_rl_repo", "/root/.axon_site/_ro/trn_rl_repo"):
    if _p not in sys.path:
        sys.path.append(_p)

import numpy as np

import concourse.bass as bass
from concourse import mybir
from concourse.bass_utils import run_bass_kernel_spmd

T, C, H, W = 16, 3, 512, 512
PS, PT = 7, 1
NCORES = 8
FPC = T // NCORES          # frames per core
NPIX = FPC * H * W         # pixels per core
NELEM = NPIX * C           # channels-last elements per core
P = 128                    # SBUF partitions
MIN_DEV_CLASS = 2          # depth-1 elements need no addition; host routes them
MERGE_W = 512              # merge layer loads narrower than this (cols)
MERGE_STORE_COLS = 256     # merge deepest regions until stores are this wide


def _prep_core(patches_k, q_k, base_k):
    """Per-core contribution stream + depth classes (host, pure indexing)."""
    h = q_k[:, 1]
    w = q_k[:, 2]
    lt = q_k[:, 0]

    dh = np.arange(PS, dtype=np.int64)
    dw = np.arange(PS, dtype=np.int64)
    ch = np.arange(C, dtype=np.int64)
    # channels-last element index, axis order (n, c, dh, dw) = patches order
    pix = (lt[:, None, None] * H + (h[:, None, None] + dh[None, :, None])) * W + (
        w[:, None, None] + dw[None, None, :]
    )
    e = (pix[:, None, :, :] * C + ch[None, :, None, None]).reshape(-1)
    v = patches_k.reshape(-1)

    if base_k is not None:
        # fold the base video in as one extra contribution per element
        e = np.concatenate([e, np.arange(NELEM, dtype=np.int64)])
        v = np.concatenate([v, base_k.reshape(-1)])

    cnt = np.bincount(e, minlength=NELEM)          # depth per element
    order = np.argsort(e, kind="stable")
    es = e[order]
    vs = v[order]
    grp_start = np.cumsum(cnt) - cnt
    rank = np.arange(es.shape[0], dtype=np.int64) - grp_start[es]

    elem_class = cnt
    max_d = int(cnt.max()) if cnt.size else 0
    class_sizes = np.bincount(elem_class, minlength=max_d + 1)
    pos_in_class = np.empty(NELEM, dtype=np.int64)
    cls_order = np.argsort(elem_class, kind="stable")
    cls_starts = np.cumsum(class_sizes) - class_sizes
    pos_in_class[cls_order] = np.arange(NELEM, dtype=np.int64) - cls_starts[
        elem_class[cls_order]
    ]
    return es, vs, rank, elem_class, pos_in_class, class_sizes


def _layout(Wl, maxd):
    """fp16 layer-major layout from per-layer column widths.

    Wl[l] (l = 1..maxd-1) is the column width of accumulation layer l =
    max over cores of ceil(#elements with depth >= l+1 / 128); Wl[0] is an
    alias of Wl[1] (the acc region covers every device element). Widths are
    non-increasing in l.

    Returns a dict with:
      W0          acc-region columns (== Wl[1])
      W[l]        layer widths
      sb_off[l]   SBUF column of layer l's staging slice (l>=1); acc at 0
      totf        total SBUF columns (acc + staging)
      BO[l]       DRAM element offset of layer l's partition-0 row (l=0..)
      RW[l]       DRAM row stride (elements) of the block holding layer l
      loads       [(dram_base, row_w, sb_col, layers)] one DMA per entry
      X           Pool-chain columns (deepest region), 0 if none
      dve_tts     [(l, width)] DVE chain: cols [X, X+width) per layer
      pool_tts    [(l, width)] Pool chain: cols [0, width) per layer
      stores      [(gates, sb_col, ncols, out_base)], gates = [(eng, cnt)]
      regions     [(a, b)] store column ranges ascending (out_base = 128a)
      vals_len    DRAM input elements (fp16)
      out_len     DRAM output elements (fp16)
    """
    W0 = Wl[1]
    sb_off = {}
    off = W0
    for l in range(1, maxd):
        sb_off[l] = off
        off += Wl[l]
    totf = off

    # --- load blocks ---
    # block0 holds layers 0+1 (every device element's first two values),
    # row-interleaved so one DMA fills acc + the first staging slice.
    BO = {0: 0, 1: W0}
    RW = {0: W0 + Wl[1], 1: W0 + Wl[1]}
    loads = [(0, W0 + Wl[1], 0, (0, 1))]
    base = 128 * (W0 + Wl[1])
    rest = [l for l in range(2, maxd)]
    head = [l for l in rest if Wl[l] >= MERGE_W]
    tail = [l for l in rest if Wl[l] < MERGE_W]
    for l in head:
        BO[l] = base
        RW[l] = Wl[l]
        loads.append((base, Wl[l], sb_off[l], (l,)))
        base += 128 * Wl[l]
    if tail:
        tw = sum(Wl[l] for l in tail)
        pref = 0
        for l in tail:
            BO[l] = base + pref
            RW[l] = tw
            pref += Wl[l]
        loads.append((base, tw, sb_off[tail[0]], tuple(tail)))
        base += 128 * tw
    vals_len = base

    # --- engine split: Pool owns the deepest columns [0, X) across all
    # layers (short, slow chain hidden under the wide DVE adds); DVE owns
    # [X, W[l]) for the layers that reach past X. Column-disjoint => no
    # cross-engine hazard. Elements at cols >= X have depth <= L* (where
    # X = Wl[L*]), so their layers all live in the DVE chain.
    X = 0
    narrow = [l for l in range(2, maxd) if Wl[l] < MERGE_STORE_COLS]
    if narrow:
        X = Wl[narrow[0]]
    dve_tts = [(l, Wl[l] - X) for l in range(1, maxd) if Wl[l] > X]
    pool_tts = [(l, min(Wl[l], X)) for l in range(1, maxd) if X > 0]

    # --- stores: column region [Wl[d], Wl[d-1]) holds (at most) the
    # depth-d elements; it is final once the layers wider than its start
    # have folded. Walk ascending d while regions stay wide; the deep
    # remainder [0, S) becomes one store gated on both full chains. ---
    def ndve(a):
        return sum(1 for l, _ in dve_tts if Wl[l] > a)

    stores = []
    regions = []
    S = W0
    for d in range(2, maxd + 1):
        hi = Wl[d - 1]
        lo = Wl[d] if d < maxd else 0
        if hi - lo < MERGE_STORE_COLS or lo < MERGE_STORE_COLS:
            S = hi
            break
        stores.append(([("dve", ndve(lo))], lo, hi - lo, 128 * lo))
        regions.append((lo, hi))
        S = lo
    if S > 0:
        gates = []
        if pool_tts:
            gates.append(("pool", len(pool_tts)))
        if dve_tts:
            gates.append(("dve", len(dve_tts)))
        stores.append((gates, 0, S, 0))
        regions.append((0, S))
    regions.sort()
    out_len = 128 * W0

    return {
        "W0": W0, "W": Wl, "sb_off": sb_off, "totf": totf,
        "BO": BO, "RW": RW, "loads": loads, "stores": stores,
        "regions": regions, "X": X, "dve_tts": dve_tts, "pool_tts": pool_tts,
        "vals_len": vals_len, "out_len": out_len, "maxd": maxd,
    }


def plan(vid2fill, patches, queryInds):
    """Host-side plan: layer widths + per-core packed fp16 values + metadata."""
    vid2fill = np.asarray(vid2fill, dtype=np.float32)
    patches = np.asarray(patches, dtype=np.float32)
    queryInds = np.asarray(queryInds, dtype=np.int64)

    base_nonzero = bool(np.any(vid2fill))
    vid_cl = np.ascontiguousarray(vid2fill.transpose(0, 2, 3, 1))  # [T,H,W,C]

    core_of = queryInds[:, 0] // FPC
    core_data = []
    for k in range(NCORES):
        sel = core_of == k
        q_k = queryInds[sel].copy()
        q_k[:, 0] -= k * FPC
        base_k = (
            vid_cl[k * FPC : (k + 1) * FPC].reshape(-1) if base_nonzero else None
        )
        core_data.append(_prep_core(patches[sel], q_k, base_k))

    maxd = max(cd[5].shape[0] - 1 for cd in core_data)
    # Nge[k][d] = #elements of core k with depth >= d; class_start[k][d] =
    # Nge[k][d+1] = sorted position where depth-d elements begin
    nge = np.zeros((NCORES, maxd + 2), dtype=np.int64)
    for k, cd in enumerate(core_data):
        cs = cd[5]
        for d in range(maxd, MIN_DEV_CLASS - 1, -1):
            nd = int(cs[d]) if d < cs.shape[0] else 0
            nge[k, d] = nd + nge[k, d + 1]
    Wl = {l: int(-(-nge[:, l + 1].max() // P)) for l in range(1, maxd)}
    Wl[0] = Wl[1]

    lay = _layout(Wl, maxd)
    BO, RW = lay["BO"], lay["RW"]

    per_core_vals = []
    per_core_meta = []
    for k, (es, vs, rank, elem_class, pos_in_class, class_sizes) in enumerate(
        core_data
    ):
        vals = np.zeros(lay["vals_len"], dtype=np.float16)
        dcls = elem_class[es]
        dev = dcls >= MIN_DEV_CLASS
        # sorted (depth-descending) position of each contribution's element
        srt = nge[k, dcls[dev] + 1] + pos_in_class[es[dev]]
        r = rank[dev]
        bo = np.zeros(r.shape[0], dtype=np.int64)
        rw = np.zeros(r.shape[0], dtype=np.int64)
        for l in range(maxd):
            lm = r == l
            if lm.any():
                bo[lm] = BO[l]
                rw[lm] = RW[l]
        vals[bo + (srt % P) * rw + srt // P] = vs[dev]
        single = dcls == 1
        per_core_vals.append(vals)
        per_core_meta.append(
            (elem_class, pos_in_class, nge[k], es[single], vs[single])
        )
    return {
        "widths": tuple(Wl[l] for l in range(maxd)),
        "maxd": maxd,
        "layout": lay,
        "per_core_vals": per_core_vals,
        "per_core_meta": per_core_meta,
        "base_nonzero": base_nonzero,
        "vid_cl": vid_cl,
    }


def build_nc(lay):
    """Raw-Bass SPMD program: per layer, one contiguous fp16 load and one
    in-place tensor_add over the layer's column prefix (DVE for cols
    [X, W[l]), Pool for [0, X)); column regions are stored as soon as the
    layers covering them have folded."""
    Wl, sb_off, totf = lay["W"], lay["sb_off"], lay["totf"]
    nc = bass.Bass()
    f16 = mybir.dt.float16
    vals_t = nc.dram_tensor("vals", [lay["vals_len"]], f16, kind="ExternalInput")
    out_t = nc.dram_tensor("out", [lay["out_len"]], f16, kind="ExternalOutput")

    X = lay["X"]
    dve_tts = lay["dve_tts"]
    pool_tts = lay["pool_tts"]
    # which load DMA (by index) supplies each layer
    load_of_layer = {}
    for i, (_, _, _, ls) in enumerate(lay["loads"]):
        for l in ls:
            load_of_layer[l] = i

    with ExitStack() as ctx:
        sb = ctx.enter_context(nc.sbuf_tensor([P, totf], f16))
        ld_sem = {
            i: ctx.enter_context(nc.semaphore(name=f"ld_sem_{i}"))
            for i in range(len(lay["loads"]))
        }
        st_sem = ctx.enter_context(nc.semaphore(name="st_sem"))
        dve_sem = ctx.enter_context(nc.semaphore(name="dve_sem"))
        pool_sem = (
            ctx.enter_context(nc.semaphore(name="pool_sem")) if pool_tts else None
        )
        block = ctx.enter_context(nc.Block())

        @block.sync
        def _(sync):
            for i, (base, row_w, sb_col, ls) in enumerate(lay["loads"]):
                src = vals_t[base : base + 128 * row_w].rearrange(
                    "(p x) -> p x", p=P
                )
                sync.dma_start(
                    sb[:, sb_col : sb_col + row_w], src
                ).then_inc(ld_sem[i], 16)
            for gates, sb_col, ncols, out_base in lay["stores"]:
                for eng, gate_cnt in gates:
                    sync.wait_ge(dve_sem if eng == "dve" else pool_sem, gate_cnt)
                dst = out_t[out_base : out_base + 128 * ncols].rearrange(
                    "(p x) -> p x", p=P
                )
                sync.dma_start(
                    dst, sb[:, sb_col : sb_col + ncols]
                ).then_inc(st_sem, 16)

        @block.vector
        def _(vector):
            for i, (l, w) in enumerate(dve_tts):
                if i > 0:
                    vector.wait_ge(dve_sem, i)  # in-place RAW chain
                vector.wait_ge(ld_sem[load_of_layer[l]], 16)
                nc.vector.tensor_add(
                    out=sb[:, X : X + w],
                    in0=sb[:, X : X + w],
                    in1=sb[:, sb_off[l] + X : sb_off[l] + X + w],
                ).then_inc(dve_sem, 1)

        if pool_tts:

            @block.gpsimd
            def _(gp):
                for i, (l, w) in enumerate(pool_tts):
                    if i > 0:
                        gp.wait_ge(pool_sem, i)  # in-place RAW chain
                    gp.wait_ge(ld_sem[load_of_layer[l]], 16)
                    nc.gpsimd.tensor_add(
                        out=sb[:, 0:w],
                        in0=sb[:, 0:w],
                        in1=sb[:, sb_off[l] : sb_off[l] + w],
                    ).then_inc(pool_sem, 1)

    return nc


_NC_CACHE = {}


def kernel(vid2fill, patches, queryInds):
    pl = plan(vid2fill, patches, queryInds)
    lay = pl["layout"]

    key = (pl["maxd"], pl["widths"])
    if key not in _NC_CACHE:
        _NC_CACHE[key] = build_nc(lay)
    nc = _NC_CACHE[key]

    in_maps = [{"vals": pl["per_core_vals"][k]} for k in range(NCORES)]
    res = run_bass_kernel_spmd(nc, in_maps, core_ids=list(range(NCORES)))

    # store region lookup tables (column -> region start/width/out_base)
    starts = np.array([a for a, b in lay["regions"]], dtype=np.int64)
    widths = np.array([b - a for a, b in lay["regions"]], dtype=np.int64)

    vid_cl = pl["vid_cl"]
    full = np.empty((T, H, W, C), dtype=np.float32)
    for k in range(NCORES):
        elem_class, pos_in_class, nge_k, single_e, single_v = pl["per_core_meta"][k]
        dev = res.results[k]["out"].astype(np.float32)
        core_out = np.empty(NELEM, dtype=np.float32)
        # depth 0: base only (with a nonzero base it was folded in, so
        # depth 0 then means a true zero — vid_cl there is what we want
        # only when the base was NOT folded; when folded, depth>=1 always)
        zero_m = elem_class == 0
        core_out[zero_m] = vid_cl[k * FPC : (k + 1) * FPC].reshape(-1)[zero_m]
        # depth 1: the single contribution, no addition needed (exact f32)
        core_out[single_e] = single_v
        # depth >= 2: device-reduced, at sorted position -> (p, col) ->
        # store-region flat offset
        dev_m = elem_class >= MIN_DEV_CLASS
        srt = nge_k[elem_class[dev_m] + 1] + pos_in_class[dev_m]
        p = srt % P
        col = srt // P
        ri = np.searchsorted(starts, col, side="right") - 1
        a = starts[ri]
        idx = 128 * a + p * widths[ri] + (col - a)
        core_out[dev_m] = dev[idx]
        full[k * FPC : (k + 1) * FPC] = core_out.reshape(FPC, H, W, C)

    return np.ascontiguousarray(full.transpose(0, 3, 1, 2))


# revision 47
# speedup vs baseline: 1.0247x; 1.0247x over previous
"""Scatter-add (col2im at random query corners) on 8 Trainium2 NeuronCores.

Problem: out[t,c,h+dh,w+dw] += patches[n,0,c,dh,dw] for each query n at
corner (t,h,w), on top of the vid2fill base. PT=1, so every patch touches
exactly one frame: shard by frame pairs (core k owns frames 2k, 2k+1); the
cores are fully independent, no collective needed.

Strategy ("depth compaction", column-major): the host computes each output
element's contributor count (its depth d) and sorts device-handled
elements (d >= 2) by depth descending; element i lands at SBUF slot
(partition i%128, column i//128). Layer r of the accumulation then only
concerns the first W[r+1] columns (a prefix), so the device streams one
contiguous fp16 DMA load per layer block and performs one in-place
full-partition vector add per layer; column regions are stored as soon as
the layers covering them have folded. Every addition of the scatter-add
happens on-device as a dense, full-bandwidth op — the memory-regime
optimum (device traffic ~= fp16 patch bytes + output bytes; padding is
under one column per layer plus cross-core size spread).

The deepest columns [0, X) are accumulated by a parallel gpsimd (Pool)
chain over all layers while the DVE chain handles [X, W[l]); the ranges
are column-disjoint so the engines never race, and the slow serial tail
of tiny deep layers runs concurrently with the wide DVE adds instead of
gating the final store. fp16 halves DMA traffic vs f32 (the 2e-2
tolerance leaves ~16x margin over fp16's ~1.2e-3 worst case); trailing
small layer loads are merged into one block and the deepest store spans
>= 256 columns so every DMA keeps >= 512 contiguous bytes per descriptor
(below that the DMA bus pays a 2x latency multiplier).

Elements with depth 0 (base only) and depth 1 (a single contribution, no
addition required anywhere) are routed by the host during unpermutation at
full f32 precision.
"""

import sys
from contextlib import ExitStack

for _p in ("/opt/trn_rl_repo", "/root/.axon_site/_ro/trn_rl_repo"):
    if _p not in sys.path:
        sys.path.append(_p)

import numpy as np

import concourse.bass as bass
from concourse import mybir
from concourse.bass_utils import run_bass_kernel_spmd

T, C, H, W = 16, 3, 512, 512
PS, PT = 7, 1
NCORES = 8
FPC = T // NCORES          # frames per core
NPIX = FPC * H * W         # pixels per core
NELEM = NPIX * C           # channels-last elements per core
P = 128                    # SBUF partitions
MIN_DEV_CLASS = 2          # depth-1 elements need no addition; host routes them
MERGE_W = 512              # merge layer loads narrower than this (cols)
MERGE_STORE_COLS = 256     # merge deepest regions until stores are this wide


def _prep_core(patches_k, q_k, base_k):
    """Per-core contribution stream + depth classes (host, pure indexing)."""
    h = q_k[:, 1]
    w = q_k[:, 2]
    lt = q_k[:, 0]

    dh = np.arange(PS, dtype=np.int64)
    dw = np.arange(PS, dtype=np.int64)
    ch = np.arange(C, dtype=np.int64)
    # channels-last element index, axis order (n, c, dh, dw) = patches order
    pix = (lt[:, None, None] * H + (h[:, None, None] + dh[None, :, None])) * W + (
        w[:, None, None] + dw[None, None, :]
    )
    e = (pix[:, None, :, :] * C + ch[None, :, None, None]).reshape(-1)
    v = patches_k.reshape(-1)

    if base_k is not None:
        # fold the base video in as one extra contribution per element
        e = np.concatenate([e, np.arange(NELEM, dtype=np.int64)])
        v = np.concatenate([v, base_k.reshape(-1)])

    cnt = np.bincount(e, minlength=NELEM)          # depth per element
    order = np.argsort(e, kind="stable")
    es = e[order]
    vs = v[order]
    grp_start = np.cumsum(cnt) - cnt
    rank = np.arange(es.shape[0], dtype=np.int64) - grp_start[es]

    elem_class = cnt
    max_d = int(cnt.max()) if cnt.size else 0
    class_sizes = np.bincount(elem_class, minlength=max_d + 1)
    pos_in_class = np.empty(NELEM, dtype=np.int64)
    cls_order = np.argsort(elem_class, kind="stable")
    cls_starts = np.cumsum(class_sizes) - class_sizes
    pos_in_class[cls_order] = np.arange(NELEM, dtype=np.int64) - cls_starts[
        elem_class[cls_order]
    ]
    return es, vs, rank, elem_class, pos_in_class, class_sizes


def _layout(Wl, maxd, scale=1.0):
    """fp16 layer-major layout from per-layer column widths.

    Wl[l] (l = 1..maxd-1) is the column width of accumulation layer l =
    max over cores of ceil(#elements with depth >= l+1 / 128); Wl[0] is an
    alias of Wl[1] (the acc region covers every device element). Widths are
    non-increasing in l.

    Returns a dict with:
      W0          acc-region columns (== Wl[1])
      W[l]        layer widths
      sb_off[l]   SBUF column of layer l's staging slice (l>=1); acc at 0
      totf        total SBUF columns (acc + staging)
      BO[l]       DRAM element offset of layer l's partition-0 row (l=0..)
      RW[l]       DRAM row stride (elements) of the block holding layer l
      loads       [(dram_base, row_w, sb_col, layers)] one DMA per entry
      X           Pool-chain columns (deepest region), 0 if none
      dve_tts     [(l, width)] DVE chain: cols [X, X+width) per layer
      pool_tts    [(l, width)] Pool chain: cols [0, width) per layer
      stores      [(gates, sb_col, ncols, out_base)], gates = [(eng, cnt)]
      regions     [(a, b)] store column ranges ascending (out_base = 128a)
      vals_len    DRAM input elements (fp16)
      out_len     DRAM output elements (fp16)
    """
    W0 = Wl[1]

    # --- engine split: Pool owns the deepest columns [0, X) across ALL
    # layers (0..maxd-1); its values travel as int8 (x scale) in one merged
    # block — Pool's op cost has no fast-dtype mode, so the narrower bytes
    # are free there, while DVE keeps the 2-byte fast mode on [X, W[l]).
    # Column-disjoint => no cross-engine hazard. Elements at cols >= X have
    # depth <= L* (where X = Wl[L*]), so their layers all live in the DVE
    # chain.
    X = 0
    narrow = [l for l in range(2, maxd) if Wl[l] < MERGE_W]
    if narrow:
        X = Wl[narrow[0]]
    pw = {l: min(Wl[l], X) for l in range(maxd)}  # pool slice per layer
    dve_tts = [(l, Wl[l] - X) for l in range(1, maxd) if Wl[l] > X]
    # pool op list: (layer, width); layer 0 is the acc-init convert
    pool_tts = [(l, pw[l]) for l in range(maxd) if X > 0 and pw[l] > 0]

    # f16 staging slices only cover [X, Wl[l]) for the DVE layers
    sb_off = {}
    off = W0
    for l, w in dve_tts:
        sb_off[l] = off
        off += w
    totf = off

    # --- load blocks (fp16, element offsets in the f16 input tensor) ---
    # block0 holds layers 0+1 over cols [X, W0), row-interleaved so one DMA
    # fills acc + the first staging slice.
    bw = W0 - X
    BO = {0: 0, 1: bw}
    RW = {0: 2 * bw, 1: 2 * bw}
    loads = [(0, 2 * bw, X, (0, 1))]
    base = 128 * 2 * bw
    for l, w in dve_tts[1:]:
        BO[l] = base
        RW[l] = w
        loads.append((base, w, sb_off[l], (l,)))
        base += 128 * w
    vals_len = base

    # --- pool block (int8): per-partition row = all layers' pool slices ---
    ppref = {}
    off = 0
    for l, w in pool_tts:
        ppref[l] = off
        off += w
    prow = off
    pool_len = 128 * prow

    # --- stores: column region [Wl[d], Wl[d-1]) holds (at most) the
    # depth-d elements; it is final once the layers wider than its start
    # have folded. Walk ascending d while regions stay wide; the deep
    # remainder [0, S) becomes one store gated on both full chains. ---
    def ndve(a):
        return sum(1 for l, _ in dve_tts if Wl[l] > a)

    stores = []
    regions = []
    S = W0
    # the DVE chain is [convert] + TTs + [rescale] when a Pool region
    # exists; store gates count positions in that chain
    cvt = 1 if pool_tts else 0
    for d in range(2, maxd + 1):
        hi = Wl[d - 1]
        lo = Wl[d] if d < maxd else 0
        if hi - lo < MERGE_STORE_COLS or lo < MERGE_STORE_COLS:
            S = hi
            break
        stores.append(([("dve", cvt + ndve(lo))], lo, hi - lo, 128 * lo))
        regions.append((lo, hi))
        S = lo
    if S > 0:
        gates = []
        if pool_tts:
            # pool chain: first op folds layers 0+1, then one add per layer
            gates.append(("pool", len(pool_tts) - 1))
            if dve_tts and S > X:
                gates.append(("dve", cvt + len(dve_tts)))
        elif dve_tts and S > X:
            gates.append(("dve", len(dve_tts)))
        stores.append((gates, 0, S, 0))
        regions.append((0, S))
    regions.sort()
    out_len = 128 * W0

    return {
        "W0": W0, "W": Wl, "sb_off": sb_off, "totf": totf,
        "BO": BO, "RW": RW, "loads": loads, "stores": stores,
        "regions": regions, "X": X, "dve_tts": dve_tts, "pool_tts": pool_tts,
        "pw": pw, "ppref": ppref, "prow": prow, "pool_len": pool_len,
        "scale": scale, "vals_len": vals_len, "out_len": out_len, "maxd": maxd,
    }


def plan(vid2fill, patches, queryInds):
    """Host-side plan: layer widths + per-core packed fp16 values + metadata."""
    vid2fill = np.asarray(vid2fill, dtype=np.float32)
    patches = np.asarray(patches, dtype=np.float32)
    queryInds = np.asarray(queryInds, dtype=np.int64)

    base_nonzero = bool(np.any(vid2fill))
    vid_cl = np.ascontiguousarray(vid2fill.transpose(0, 2, 3, 1))  # [T,H,W,C]

    core_of = queryInds[:, 0] // FPC
    core_data = []
    for k in range(NCORES):
        sel = core_of == k
        q_k = queryInds[sel].copy()
        q_k[:, 0] -= k * FPC
        base_k = (
            vid_cl[k * FPC : (k + 1) * FPC].reshape(-1) if base_nonzero else None
        )
        core_data.append(_prep_core(patches[sel], q_k, base_k))

    maxd = max(cd[5].shape[0] - 1 for cd in core_data)
    # Nge[k][d] = #elements of core k with depth >= d; class_start[k][d] =
    # Nge[k][d+1] = sorted position where depth-d elements begin
    nge = np.zeros((NCORES, maxd + 2), dtype=np.int64)
    for k, cd in enumerate(core_data):
        cs = cd[5]
        for d in range(maxd, MIN_DEV_CLASS - 1, -1):
            nd = int(cs[d]) if d < cs.shape[0] else 0
            nge[k, d] = nd + nge[k, d + 1]
    Wl = {l: int(-(-nge[:, l + 1].max() // P)) for l in range(1, maxd)}
    Wl[0] = Wl[1]

    # global int8 scale for the Pool region (quantization of deep-element
    # values; the error stays well inside the 2e-2 gate — measured 1.05e-2
    # even with every value quantized)
    amax = float(np.abs(patches).max())
    if base_nonzero:
        amax = max(amax, float(np.abs(vid_cl).max()))
    scale = max(amax, 1e-30) / 127.0

    lay = _layout(Wl, maxd, scale)
    BO, RW, X = lay["BO"], lay["RW"], lay["X"]
    pwv = np.zeros(maxd, dtype=np.int64)
    pprefv = np.zeros(maxd, dtype=np.int64)
    for l in range(maxd):
        pwv[l] = lay["pw"].get(l, 0)
        pprefv[l] = lay["ppref"].get(l, 0)

    per_core_vals = []
    per_core_meta = []
    for k, (es, vs, rank, elem_class, pos_in_class, class_sizes) in enumerate(
        core_data
    ):
        vals = np.zeros(lay["vals_len"], dtype=np.float16)
        pvals = np.zeros(max(lay["pool_len"], 1), dtype=np.int8)
        dcls = elem_class[es]
        dev = dcls >= MIN_DEV_CLASS
        # sorted (depth-descending) position of each contribution's element
        srt = nge[k, dcls[dev] + 1] + pos_in_class[es[dev]]
        r = rank[dev]
        v = vs[dev]
        p = srt % P
        col = srt // P
        in_pool = col < pwv[r]
        # pool block: int8 at pbase-relative p*prow + ppref[layer] + col
        if lay["pool_len"]:
            q = np.clip(np.round(v[in_pool] / scale), -127, 127).astype(np.int8)
            pvals[
                p[in_pool] * lay["prow"] + pprefv[r[in_pool]] + col[in_pool]
            ] = q
        # f16 blocks: cols shifted by X
        fm = ~in_pool
        bo = np.zeros(int(fm.sum()), dtype=np.int64)
        rw = np.zeros(int(fm.sum()), dtype=np.int64)
        rf = r[fm]
        for l in range(maxd):
            lm = rf == l
            if lm.any():
                bo[lm] = BO[l]
                rw[lm] = RW[l]
        vals[bo + p[fm] * rw + (col[fm] - X)] = v[fm]
        single = dcls == 1
        per_core_vals.append((vals, pvals))
        per_core_meta.append(
            (elem_class, pos_in_class, nge[k], es[single], vs[single])
        )
    return {
        "widths": tuple(Wl[l] for l in range(maxd)),
        "maxd": maxd,
        "layout": lay,
        "per_core_vals": per_core_vals,
        "per_core_meta": per_core_meta,
        "base_nonzero": base_nonzero,
        "vid_cl": vid_cl,
    }


def build_nc(lay):
    """Raw-Bass SPMD program: per layer, one contiguous fp16 load and one
    in-place tensor_add over the layer's column prefix (DVE for cols
    [X, W[l]), Pool for [0, X)); column regions are stored as soon as the
    layers covering them have folded."""
    Wl, sb_off, totf = lay["W"], lay["sb_off"], lay["totf"]
    nc = bass.Bass()
    f16 = mybir.dt.float16
    i8 = mybir.dt.int8
    vals_t = nc.dram_tensor("vals", [lay["vals_len"]], f16, kind="ExternalInput")
    pvals_t = nc.dram_tensor(
        "pvals", [max(lay["pool_len"], 1)], i8, kind="ExternalInput"
    )
    out_t = nc.dram_tensor("out", [lay["out_len"]], f16, kind="ExternalOutput")

    X = lay["X"]
    s = lay["scale"]
    dve_tts = lay["dve_tts"]
    pool_tts = lay["pool_tts"]
    ppref, prow = lay["ppref"], lay["prow"]
    mult, add = mybir.AluOpType.mult, mybir.AluOpType.add
    # which f16 load DMA (by index) supplies each DVE layer
    load_of_layer = {}
    for i, (_, _, _, ls) in enumerate(lay["loads"]):
        for l in ls:
            load_of_layer[l] = i

    with ExitStack() as ctx:
        sb = ctx.enter_context(nc.sbuf_tensor("sb", [P, totf], f16))
        pb = (
            ctx.enter_context(nc.sbuf_tensor("pb", [P, prow], i8))
            if pool_tts
            else None
        )
        pf = (
            ctx.enter_context(nc.sbuf_tensor("pf", [P, prow], f16))
            if pool_tts
            else None
        )
        ld_sem = {
            i: ctx.enter_context(nc.semaphore(name=f"ld_sem_{i}"))
            for i in range(len(lay["loads"]))
        }
        pl_sem = (
            ctx.enter_context(nc.semaphore(name="pl_sem")) if pool_tts else None
        )
        st_sem = ctx.enter_context(nc.semaphore(name="st_sem"))
        dve_sem = ctx.enter_context(nc.semaphore(name="dve_sem"))
        pool_sem = (
            ctx.enter_context(nc.semaphore(name="pool_sem")) if pool_tts else None
        )
        block = ctx.enter_context(nc.Block())

        @block.sync
        def _(sync):
            if pool_tts:
                # int8 pool block first: it is tiny and unblocks the whole
                # Pool chain at once
                src = pvals_t[0 : 128 * prow].rearrange("(p x) -> p x", p=P)
                sync.dma_start(pb[:, 0:prow], src).then_inc(pl_sem, 16)
            for i, (base, row_w, sb_col, ls) in enumerate(lay["loads"]):
                src = vals_t[base : base + 128 * row_w].rearrange(
                    "(p x) -> p x", p=P
                )
                sync.dma_start(
                    sb[:, sb_col : sb_col + row_w], src
                ).then_inc(ld_sem[i], 16)
            for gates, sb_col, ncols, out_base in lay["stores"]:
                for eng, gate_cnt in gates:
                    sync.wait_ge(dve_sem if eng == "dve" else pool_sem, gate_cnt)
                dst = out_t[out_base : out_base + 128 * ncols].rearrange(
                    "(p x) -> p x", p=P
                )
                sync.dma_start(
                    dst, sb[:, sb_col : sb_col + ncols]
                ).then_inc(st_sem, 16)

        @block.vector
        def _(vector):
            if pool_tts:
                # dequantize the int8 block into f16 staging during DVE's
                # pre-block0 idle: one op converts AND applies the scale
                vector.wait_ge(pl_sem, 16)
                nc.vector.tensor_scalar_mul(
                    out=pf[:, 0:prow], in0=pb[:, 0:prow], scalar1=s
                ).then_inc(dve_sem, 1)
            base_cnt = 1 if pool_tts else 0
            for i, (l, w) in enumerate(dve_tts):
                if i > 0:
                    vector.wait_ge(dve_sem, base_cnt + i)  # in-place RAW
                vector.wait_ge(ld_sem[load_of_layer[l]], 16)
                nc.vector.tensor_add(
                    out=sb[:, X : X + w],
                    in0=sb[:, X : X + w],
                    in1=sb[:, sb_off[l] : sb_off[l] + w],
                ).then_inc(dve_sem, 1)

        if pool_tts:

            @block.gpsimd
            def _(gp):
                # wait for the convert AND the first DVE TT: the extra op
                # puts ~14us between the convert's sem inc and this read,
                # clearing the cross-engine SBUF write-commit window (gating
                # on the inc alone raced on hardware: nondeterministic
                # stale reads of the staging tail)
                gp.wait_ge(dve_sem, min(2, 1 + len(dve_tts)))
                first, second = pool_tts[0], pool_tts[1]
                nc.gpsimd.tensor_add(
                    out=sb[:, 0 : first[1]],
                    in0=pf[:, ppref[first[0]] : ppref[first[0]] + first[1]],
                    in1=pf[:, ppref[second[0]] : ppref[second[0]] + second[1]],
                ).then_inc(pool_sem, 1)
                for i, (l, w) in enumerate(pool_tts[2:]):
                    gp.wait_ge(pool_sem, i + 1)  # in-place RAW chain
                    nc.gpsimd.tensor_add(
                        out=sb[:, 0:w],
                        in0=sb[:, 0:w],
                        in1=pf[:, ppref[l] : ppref[l] + w],
                    ).then_inc(pool_sem, 1)

    return nc


_NC_CACHE = {}


def kernel(vid2fill, patches, queryInds):
    pl = plan(vid2fill, patches, queryInds)
    lay = pl["layout"]

    key = (pl["maxd"], pl["widths"], lay["scale"])
    if key not in _NC_CACHE:
        _NC_CACHE[key] = build_nc(lay)
    nc = _NC_CACHE[key]

    in_maps = [
        {"vals": pl["per_core_vals"][k][0], "pvals": pl["per_core_vals"][k][1]}
        for k in range(NCORES)
    ]
    res = run_bass_kernel_spmd(nc, in_maps, core_ids=list(range(NCORES)))

    # store region lookup tables (column -> region start/width/out_base)
    starts = np.array([a for a, b in lay["regions"]], dtype=np.int64)
    widths = np.array([b - a for a, b in lay["regions"]], dtype=np.int64)

    vid_cl = pl["vid_cl"]
    full = np.empty((T, H, W, C), dtype=np.float32)
    for k in range(NCORES):
        elem_class, pos_in_class, nge_k, single_e, single_v = pl["per_core_meta"][k]
        dev = res.results[k]["out"].astype(np.float32)
        core_out = np.empty(NELEM, dtype=np.float32)
        # depth 0: base only (with a nonzero base it was folded in, so
        # depth 0 then means a true zero — vid_cl there is what we want
        # only when the base was NOT folded; when folded, depth>=1 always)
        zero_m = elem_class == 0
        core_out[zero_m] = vid_cl[k * FPC : (k + 1) * FPC].reshape(-1)[zero_m]
        # depth 1: the single contribution, no addition needed (exact f32)
        core_out[single_e] = single_v
        # depth >= 2: device-reduced, at sorted position -> (p, col) ->
        # store-region flat offset
        dev_m = elem_class >= MIN_DEV_CLASS
        srt = nge_k[elem_class[dev_m] + 1] + pos_in_class[dev_m]
        p = srt % P
        col = srt // P
        ri = np.searchsorted(starts, col, side="right") - 1
        a = starts[ri]
        idx = 128 * a + p * widths[ri] + (col - a)
        core_out[dev_m] = dev[idx]
        full[k * FPC : (k + 1) * FPC] = core_out.reshape(FPC, H, W, C)

    return np.ascontiguousarray(full.transpose(0, 3, 1, 2))


# revision 48
# speedup vs baseline: 1.0319x; 1.0070x over previous
"""Scatter-add (col2im at random query corners) on 8 Trainium2 NeuronCores.

Problem: out[t,c,h+dh,w+dw] += patches[n,0,c,dh,dw] for each query n at
corner (t,h,w), on top of the vid2fill base. PT=1, so every patch touches
exactly one frame: shard by frame pairs (core k owns frames 2k, 2k+1); the
cores are fully independent, no collective needed.

Strategy ("depth compaction", column-major): the host computes each output
element's contributor count (its depth d) and sorts device-handled
elements (d >= 2) by depth descending; element i lands at SBUF slot
(partition i%128, column i//128). Layer r of the accumulation then only
concerns the first W[r+1] columns (a prefix), so the device streams one
contiguous fp16 DMA load per layer block and performs one in-place
full-partition vector add per layer; column regions are stored as soon as
the layers covering them have folded. Every addition of the scatter-add
happens on-device as a dense, full-bandwidth op — the memory-regime
optimum (device traffic ~= fp16 patch bytes + output bytes; padding is
under one column per layer plus cross-core size spread).

The deepest columns [0, X) are accumulated by a parallel gpsimd (Pool)
chain over all layers while the DVE chain handles [X, W[l]); the ranges
are column-disjoint so the engines never race, and the slow serial tail
of tiny deep layers runs concurrently with the wide DVE adds instead of
gating the final store. fp16 halves DMA traffic vs f32 (the 2e-2
tolerance leaves ~16x margin over fp16's ~1.2e-3 worst case); trailing
small layer loads are merged into one block and the deepest store spans
>= 256 columns so every DMA keeps >= 512 contiguous bytes per descriptor
(below that the DMA bus pays a 2x latency multiplier).

Elements with depth 0 (base only) and depth 1 (a single contribution, no
addition required anywhere) are routed by the host during unpermutation at
full f32 precision.
"""

import sys
from contextlib import ExitStack

for _p in ("/opt/trn_rl_repo", "/root/.axon_site/_ro/trn_rl_repo"):
    if _p not in sys.path:
        sys.path.append(_p)

import numpy as np

import concourse.bass as bass
from concourse import mybir
from concourse.bass_utils import run_bass_kernel_spmd

T, C, H, W = 16, 3, 512, 512
PS, PT = 7, 1
NCORES = 8
FPC = T // NCORES          # frames per core
NPIX = FPC * H * W         # pixels per core
NELEM = NPIX * C           # channels-last elements per core
P = 128                    # SBUF partitions
MIN_DEV_CLASS = 2          # depth-1 elements need no addition; host routes them
MERGE_W = 512              # merge layer loads narrower than this (cols)
MERGE_STORE_COLS = 256     # merge deepest regions until stores are this wide


def _prep_core(patches_k, q_k, base_k):
    """Per-core contribution stream + depth classes (host, pure indexing)."""
    h = q_k[:, 1]
    w = q_k[:, 2]
    lt = q_k[:, 0]

    dh = np.arange(PS, dtype=np.int64)
    dw = np.arange(PS, dtype=np.int64)
    ch = np.arange(C, dtype=np.int64)
    # channels-last element index, axis order (n, c, dh, dw) = patches order
    pix = (lt[:, None, None] * H + (h[:, None, None] + dh[None, :, None])) * W + (
        w[:, None, None] + dw[None, None, :]
    )
    e = (pix[:, None, :, :] * C + ch[None, :, None, None]).reshape(-1)
    v = patches_k.reshape(-1)

    if base_k is not None:
        # fold the base video in as one extra contribution per element
        e = np.concatenate([e, np.arange(NELEM, dtype=np.int64)])
        v = np.concatenate([v, base_k.reshape(-1)])

    cnt = np.bincount(e, minlength=NELEM)          # depth per element
    order = np.argsort(e, kind="stable")
    es = e[order]
    vs = v[order]
    grp_start = np.cumsum(cnt) - cnt
    rank = np.arange(es.shape[0], dtype=np.int64) - grp_start[es]

    elem_class = cnt
    max_d = int(cnt.max()) if cnt.size else 0
    class_sizes = np.bincount(elem_class, minlength=max_d + 1)
    pos_in_class = np.empty(NELEM, dtype=np.int64)
    cls_order = np.argsort(elem_class, kind="stable")
    cls_starts = np.cumsum(class_sizes) - class_sizes
    pos_in_class[cls_order] = np.arange(NELEM, dtype=np.int64) - cls_starts[
        elem_class[cls_order]
    ]
    return es, vs, rank, elem_class, pos_in_class, class_sizes


def _layout(Wl, maxd, scale=1.0):
    """fp16 layer-major layout from per-layer column widths.

    Wl[l] (l = 1..maxd-1) is the column width of accumulation layer l =
    max over cores of ceil(#elements with depth >= l+1 / 128); Wl[0] is an
    alias of Wl[1] (the acc region covers every device element). Widths are
    non-increasing in l.

    Returns a dict with:
      W0          acc-region columns (== Wl[1])
      W[l]        layer widths
      sb_off[l]   SBUF column of layer l's staging slice (l>=1); acc at 0
      totf        total SBUF columns (acc + staging)
      BO[l]       DRAM element offset of layer l's partition-0 row (l=0..)
      RW[l]       DRAM row stride (elements) of the block holding layer l
      loads       [(dram_base, row_w, sb_col, layers)] one DMA per entry
      X           Pool-chain columns (deepest region), 0 if none
      dve_tts     [(l, width)] DVE chain: cols [X, X+width) per layer
      pool_tts    [(l, width)] Pool chain: cols [0, width) per layer
      stores      [(gates, sb_col, ncols, out_base)], gates = [(eng, cnt)]
      regions     [(a, b)] store column ranges ascending (out_base = 128a)
      vals_len    DRAM input elements (fp16)
      out_len     DRAM output elements (fp16)
    """
    W0 = Wl[1]

    # --- engine split: Pool owns the deepest columns [0, X) across ALL
    # layers (0..maxd-1); its values travel as int8 (x scale) in one merged
    # block — Pool's op cost has no fast-dtype mode, so the narrower bytes
    # are free there, while DVE keeps the 2-byte fast mode on [X, W[l]).
    # Column-disjoint => no cross-engine hazard. Elements at cols >= X have
    # depth <= L* (where X = Wl[L*]), so their layers all live in the DVE
    # chain.
    X = 0
    narrow = [l for l in range(2, maxd) if Wl[l] < MERGE_W]
    if narrow:
        X = Wl[narrow[0]]
    pw = {l: min(Wl[l], X) for l in range(maxd)}  # pool slice per layer
    dve_tts = [(l, Wl[l] - X) for l in range(1, maxd) if Wl[l] > X]
    # pool op list: (layer, width); layer 0 is the acc-init convert
    pool_tts = [(l, pw[l]) for l in range(maxd) if X > 0 and pw[l] > 0]

    # f16 staging slices only cover [X, Wl[l]) for the DVE layers
    sb_off = {}
    off = W0
    for l, w in dve_tts:
        sb_off[l] = off
        off += w
    totf = off

    # --- load blocks (fp16, element offsets in the f16 input tensor) ---
    # block0 holds layers 0+1 over cols [X, W0), row-interleaved so one DMA
    # fills acc + the first staging slice.
    bw = W0 - X
    BO = {0: 0, 1: bw}
    RW = {0: 2 * bw, 1: 2 * bw}
    loads = [(0, 2 * bw, X, (0, 1))]
    base = 128 * 2 * bw
    for l, w in dve_tts[1:]:
        BO[l] = base
        RW[l] = w
        loads.append((base, w, sb_off[l], (l,)))
        base += 128 * w
    vals_len = base

    # --- pool block (int8): per-partition row = all layers' pool slices ---
    ppref = {}
    off = 0
    for l, w in pool_tts:
        ppref[l] = off
        off += w
    prow = off
    pool_len = 128 * prow

    # --- stores: column region [Wl[d], Wl[d-1]) holds (at most) the
    # depth-d elements; it is final once the layers wider than its start
    # have folded. Walk ascending d while regions stay wide; the deep
    # remainder [0, S) becomes one store gated on both full chains. ---
    def ndve(a):
        return sum(1 for l, _ in dve_tts if Wl[l] > a)

    stores = []
    regions = []
    S = W0
    # the DVE chain is [convert] + TTs + [rescale] when a Pool region
    # exists; store gates count positions in that chain
    cvt = 1 if pool_tts else 0
    for d in range(2, maxd + 1):
        hi = Wl[d - 1]
        lo = Wl[d] if d < maxd else 0
        if hi - lo < MERGE_STORE_COLS or lo < MERGE_STORE_COLS:
            S = hi
            break
        stores.append(([("dve", cvt + ndve(lo))], lo, hi - lo, 128 * lo))
        regions.append((lo, hi))
        S = lo
    if S > 0:
        gates = []
        if pool_tts:
            # pool chain: first op folds layers 0+1, then one add per layer
            gates.append(("pool", len(pool_tts) - 1))
            if dve_tts and S > X:
                gates.append(("dve", cvt + len(dve_tts)))
        elif dve_tts and S > X:
            gates.append(("dve", len(dve_tts)))
        stores.append((gates, 0, S, 0))
        regions.append((0, S))
    regions.sort()
    out_len = 128 * W0

    return {
        "W0": W0, "W": Wl, "sb_off": sb_off, "totf": totf,
        "BO": BO, "RW": RW, "loads": loads, "stores": stores,
        "regions": regions, "X": X, "dve_tts": dve_tts, "pool_tts": pool_tts,
        "pw": pw, "ppref": ppref, "prow": prow, "pool_len": pool_len,
        "scale": scale, "vals_len": vals_len, "out_len": out_len, "maxd": maxd,
    }


def plan(vid2fill, patches, queryInds):
    """Host-side plan: layer widths + per-core packed fp16 values + metadata."""
    vid2fill = np.asarray(vid2fill, dtype=np.float32)
    patches = np.asarray(patches, dtype=np.float32)
    queryInds = np.asarray(queryInds, dtype=np.int64)

    base_nonzero = bool(np.any(vid2fill))
    vid_cl = np.ascontiguousarray(vid2fill.transpose(0, 2, 3, 1))  # [T,H,W,C]

    core_of = queryInds[:, 0] // FPC
    core_data = []
    for k in range(NCORES):
        sel = core_of == k
        q_k = queryInds[sel].copy()
        q_k[:, 0] -= k * FPC
        base_k = (
            vid_cl[k * FPC : (k + 1) * FPC].reshape(-1) if base_nonzero else None
        )
        core_data.append(_prep_core(patches[sel], q_k, base_k))

    maxd = max(cd[5].shape[0] - 1 for cd in core_data)
    # Nge[k][d] = #elements of core k with depth >= d; class_start[k][d] =
    # Nge[k][d+1] = sorted position where depth-d elements begin
    nge = np.zeros((NCORES, maxd + 2), dtype=np.int64)
    for k, cd in enumerate(core_data):
        cs = cd[5]
        for d in range(maxd, MIN_DEV_CLASS - 1, -1):
            nd = int(cs[d]) if d < cs.shape[0] else 0
            nge[k, d] = nd + nge[k, d + 1]
    Wl = {l: int(-(-nge[:, l + 1].max() // P)) for l in range(1, maxd)}
    Wl[0] = Wl[1]

    # global int8 scale for the Pool region (quantization of deep-element
    # values; the error stays well inside the 2e-2 gate — measured 1.05e-2
    # even with every value quantized)
    amax = float(np.abs(patches).max())
    if base_nonzero:
        amax = max(amax, float(np.abs(vid_cl).max()))
    scale = max(amax, 1e-30) / 127.0

    lay = _layout(Wl, maxd, scale)
    BO, RW, X = lay["BO"], lay["RW"], lay["X"]
    pwv = np.zeros(maxd, dtype=np.int64)
    pprefv = np.zeros(maxd, dtype=np.int64)
    for l in range(maxd):
        pwv[l] = lay["pw"].get(l, 0)
        pprefv[l] = lay["ppref"].get(l, 0)

    per_core_vals = []
    per_core_meta = []
    for k, (es, vs, rank, elem_class, pos_in_class, class_sizes) in enumerate(
        core_data
    ):
        vals = np.zeros(lay["vals_len"], dtype=np.float16)
        pvals = np.zeros(max(lay["pool_len"], 1), dtype=np.int8)
        dcls = elem_class[es]
        dev = dcls >= MIN_DEV_CLASS
        # sorted (depth-descending) position of each contribution's element
        srt = nge[k, dcls[dev] + 1] + pos_in_class[es[dev]]
        r = rank[dev]
        v = vs[dev]
        p = srt % P
        col = srt // P
        in_pool = col < pwv[r]
        # pool block: int8 at pbase-relative p*prow + ppref[layer] + col
        if lay["pool_len"]:
            q = np.clip(np.round(v[in_pool] / scale), -127, 127).astype(np.int8)
            pvals[
                p[in_pool] * lay["prow"] + pprefv[r[in_pool]] + col[in_pool]
            ] = q
        # f16 blocks: cols shifted by X
        fm = ~in_pool
        bo = np.zeros(int(fm.sum()), dtype=np.int64)
        rw = np.zeros(int(fm.sum()), dtype=np.int64)
        rf = r[fm]
        for l in range(maxd):
            lm = rf == l
            if lm.any():
                bo[lm] = BO[l]
                rw[lm] = RW[l]
        vals[bo + p[fm] * rw + (col[fm] - X)] = v[fm]
        single = dcls == 1
        per_core_vals.append((vals, pvals))
        per_core_meta.append(
            (elem_class, pos_in_class, nge[k], es[single], vs[single])
        )
    return {
        "widths": tuple(Wl[l] for l in range(maxd)),
        "maxd": maxd,
        "layout": lay,
        "per_core_vals": per_core_vals,
        "per_core_meta": per_core_meta,
        "base_nonzero": base_nonzero,
        "vid_cl": vid_cl,
    }


def build_nc(lay):
    """Raw-Bass SPMD program: per layer, one contiguous fp16 load and one
    in-place tensor_add over the layer's column prefix (DVE for cols
    [X, W[l]), Pool for [0, X)); column regions are stored as soon as the
    layers covering them have folded."""
    Wl, sb_off, totf = lay["W"], lay["sb_off"], lay["totf"]
    nc = bass.Bass()
    f16 = mybir.dt.float16
    i8 = mybir.dt.int8
    vals_t = nc.dram_tensor("vals", [lay["vals_len"]], f16, kind="ExternalInput")
    pvals_t = nc.dram_tensor(
        "pvals", [max(lay["pool_len"], 1)], i8, kind="ExternalInput"
    )
    out_t = nc.dram_tensor("out", [lay["out_len"]], f16, kind="ExternalOutput")

    X = lay["X"]
    s = lay["scale"]
    dve_tts = lay["dve_tts"]
    pool_tts = lay["pool_tts"]
    ppref, prow = lay["ppref"], lay["prow"]
    mult, add = mybir.AluOpType.mult, mybir.AluOpType.add
    # which f16 load DMA (by index) supplies each DVE layer
    load_of_layer = {}
    for i, (_, _, _, ls) in enumerate(lay["loads"]):
        for l in ls:
            load_of_layer[l] = i

    with ExitStack() as ctx:
        sb = ctx.enter_context(nc.sbuf_tensor("sb", [P, totf], f16))
        pb = (
            ctx.enter_context(nc.sbuf_tensor("pb", [P, prow], i8))
            if pool_tts
            else None
        )
        pf = (
            ctx.enter_context(nc.sbuf_tensor("pf", [P, prow], f16))
            if pool_tts
            else None
        )
        ld_sem = {
            i: ctx.enter_context(nc.semaphore(name=f"ld_sem_{i}"))
            for i in range(len(lay["loads"]))
        }
        pl_sem = (
            ctx.enter_context(nc.semaphore(name="pl_sem")) if pool_tts else None
        )
        st_sem = ctx.enter_context(nc.semaphore(name="st_sem"))
        dve_sem = ctx.enter_context(nc.semaphore(name="dve_sem"))
        pool_sem = (
            ctx.enter_context(nc.semaphore(name="pool_sem")) if pool_tts else None
        )
        block = ctx.enter_context(nc.Block())

        @block.sync
        def _(sync):
            if pool_tts:
                # int8 pool block first: it is tiny and unblocks the whole
                # Pool chain at once
                src = pvals_t[0 : 128 * prow].rearrange("(p x) -> p x", p=P)
                sync.dma_start(pb[:, 0:prow], src).then_inc(pl_sem, 16)
            for i, (base, row_w, sb_col, ls) in enumerate(lay["loads"]):
                src = vals_t[base : base + 128 * row_w].rearrange(
                    "(p x) -> p x", p=P
                )
                sync.dma_start(
                    sb[:, sb_col : sb_col + row_w], src
                ).then_inc(ld_sem[i], 16)
            for gates, sb_col, ncols, out_base in lay["stores"]:
                for eng, gate_cnt in gates:
                    sync.wait_ge(dve_sem if eng == "dve" else pool_sem, gate_cnt)
                dst = out_t[out_base : out_base + 128 * ncols].rearrange(
                    "(p x) -> p x", p=P
                )
                sync.dma_start(
                    dst, sb[:, sb_col : sb_col + ncols]
                ).then_inc(st_sem, 16)

        @block.vector
        def _(vector):
            if pool_tts:
                # dequantize the int8 block into f16 staging during DVE's
                # pre-block0 idle: one op converts AND applies the scale
                vector.wait_ge(pl_sem, 16)
                nc.vector.tensor_scalar_mul(
                    out=pf[:, 0:prow], in0=pb[:, 0:prow], scalar1=s
                ).then_inc(dve_sem, 1)
            base_cnt = 1 if pool_tts else 0
            for i, (l, w) in enumerate(dve_tts):
                if i > 0:
                    vector.wait_ge(dve_sem, base_cnt + i)  # in-place RAW
                vector.wait_ge(ld_sem[load_of_layer[l]], 16)
                nc.vector.tensor_add(
                    out=sb[:, X : X + w],
                    in0=sb[:, X : X + w],
                    in1=sb[:, sb_off[l] : sb_off[l] + w],
                ).then_inc(dve_sem, 1)

        if pool_tts:

            @block.gpsimd
            def _(gp):
                # wait for the convert AND the (much later) block0 load: the
                # second wait puts several microseconds between the convert's
                # sem inc and this read, clearing the cross-engine SBUF
                # write-commit window (gating on the inc alone raced on
                # hardware: nondeterministic stale reads of the staging tail)
                gp.wait_ge(dve_sem, 1)
                gp.wait_ge(ld_sem[0], 16)
                first, second = pool_tts[0], pool_tts[1]
                nc.gpsimd.tensor_add(
                    out=sb[:, 0 : first[1]],
                    in0=pf[:, ppref[first[0]] : ppref[first[0]] + first[1]],
                    in1=pf[:, ppref[second[0]] : ppref[second[0]] + second[1]],
                ).then_inc(pool_sem, 1)
                for i, (l, w) in enumerate(pool_tts[2:]):
                    gp.wait_ge(pool_sem, i + 1)  # in-place RAW chain
                    nc.gpsimd.tensor_add(
                        out=sb[:, 0:w],
                        in0=sb[:, 0:w],
                        in1=pf[:, ppref[l] : ppref[l] + w],
                    ).then_inc(pool_sem, 1)

    return nc


_NC_CACHE = {}


def kernel(vid2fill, patches, queryInds):
    pl = plan(vid2fill, patches, queryInds)
    lay = pl["layout"]

    key = (pl["maxd"], pl["widths"], lay["scale"])
    if key not in _NC_CACHE:
        _NC_CACHE[key] = build_nc(lay)
    nc = _NC_CACHE[key]

    in_maps = [
        {"vals": pl["per_core_vals"][k][0], "pvals": pl["per_core_vals"][k][1]}
        for k in range(NCORES)
    ]
    res = run_bass_kernel_spmd(nc, in_maps, core_ids=list(range(NCORES)))

    # store region lookup tables (column -> region start/width/out_base)
    starts = np.array([a for a, b in lay["regions"]], dtype=np.int64)
    widths = np.array([b - a for a, b in lay["regions"]], dtype=np.int64)

    vid_cl = pl["vid_cl"]
    full = np.empty((T, H, W, C), dtype=np.float32)
    for k in range(NCORES):
        elem_class, pos_in_class, nge_k, single_e, single_v = pl["per_core_meta"][k]
        dev = res.results[k]["out"].astype(np.float32)
        core_out = np.empty(NELEM, dtype=np.float32)
        # depth 0: base only (with a nonzero base it was folded in, so
        # depth 0 then means a true zero — vid_cl there is what we want
        # only when the base was NOT folded; when folded, depth>=1 always)
        zero_m = elem_class == 0
        core_out[zero_m] = vid_cl[k * FPC : (k + 1) * FPC].reshape(-1)[zero_m]
        # depth 1: the single contribution, no addition needed (exact f32)
        core_out[single_e] = single_v
        # depth >= 2: device-reduced, at sorted position -> (p, col) ->
        # store-region flat offset
        dev_m = elem_class >= MIN_DEV_CLASS
        srt = nge_k[elem_class[dev_m] + 1] + pos_in_class[dev_m]
        p = srt % P
        col = srt // P
        ri = np.searchsorted(starts, col, side="right") - 1
        a = starts[ri]
        idx = 128 * a + p * widths[ri] + (col - a)
        core_out[dev_m] = dev[idx]
        full[k * FPC : (k + 1) * FPC] = core_out.reshape(FPC, H, W, C)

    return np.ascontiguousarray(full.transpose(0, 3, 1, 2))
